# revision 1
# baseline (speedup 1.0000x reference)
"""Trainium2 Bass kernel for nn_Attention_4329327034558.

Multi-head attention: x [4, 256, 2048], w_qkv [1536, 256], w_out [256, 512],
b_out [256] -> y [4, 256, 2048]  (8 heads, head dim 64).

Sharding over 8 NeuronCores: core c handles batch c//2 and query-half c%2
(all 8 heads). k/v are computed per core for the full sequence; q only for the
core's query half. Host side: transpose weights once, slice x per core, and
concatenate the two output halves per batch (no cross-core reduction needed).

Per-core device algorithm (attention matmuls in float16 — same accuracy as
float32r here but ~18% faster since fp16 weight loads use the fast path;
projections in float32r):
  k  = w_k @ x_b          [512, 2048]  (head-dim-major, heads stacked)
  vT = x_b^T @ w_v^T      [2048, 65*8] (produced directly transposed; a ones
                                        column is appended per head tile)
  q  = w_q @ x_b[:, half] [512, 1024]
  per head h, per key tile jt (128 keys):
    sim_T[j, i] = k_h^T q_h                   (PE, K=64 -> psum [128, 1024])
    E = exp(scale * sim_T)                    (ACT, psum -> sbuf f32r)
    [out_T | denom] += [v_h^T | 1]^T E        (PE, K=128, psum accum over jt;
                                               row 64 accumulates the softmax
                                               denominator for free)
  outn = out_T * (1/denom)   (DVE reciprocal + GPSIMD partition_broadcast +
                              DVE multiply; softmax max-subtraction is skipped:
                              logits are ~N(0,1) so exp() is safe in f32 and
                              mathematically identical to the reference)
  y_half = w_out @ concat_h(outn) + b_out     (PE + DVE bias-add)
"""

import numpy as np

import concourse.mybir as mybir
import concourse.tile as tile
from concourse import bacc
from concourse.bass_utils import run_bass_kernel_spmd

F32 = mybir.dt.float32
F32R = mybir.dt.float32r
F16 = mybir.dt.float16
AF = mybir.ActivationFunctionType

B = 4          # batch
DIM = 256      # channels
N = 2048       # sequence length
NH = 1024      # queries per core (n/2)
H = 8          # heads
DH = 64        # head dim
HID = 512      # h*dh
SCALE = DH ** -0.5
N_CORES = 8

JT = N // 128        # 16 key tiles
IC = NH // 512       # 2 query chunks


def _build_nc(num_devices=N_CORES, repeat=1):
    nc = bacc.Bacc("TRN2", target_bir_lowering=False, debug=False,
                   num_devices=num_devices)

    x_kv = nc.dram_tensor("x_kv", [DIM, N], F32, kind="ExternalInput")
    x_q = nc.dram_tensor("x_q", [DIM, NH], F32, kind="ExternalInput")
    wqT = nc.dram_tensor("wqT", [DIM, HID], F32, kind="ExternalInput")
    wkvT = nc.dram_tensor("wkvT", [DIM, 2 * HID], F32, kind="ExternalInput")
    woutT = nc.dram_tensor("woutT", [HID, DIM], F32, kind="ExternalInput")
    bout = nc.dram_tensor("bout", [128, 2], F32, kind="ExternalInput")
    y = nc.dram_tensor("y", [DIM, NH], F32, kind="ExternalOutput")

    with tile.TileContext(nc) as tc:
        with (
            tc.tile_pool(name="const", bufs=1) as cpool,
            tc.tile_pool(name="xin", bufs=1) as xpool,
            tc.tile_pool(name="kq", bufs=1) as kqpool,
            tc.tile_pool(name="epool", bufs=3) as epool,
            tc.tile_pool(name="rpool", bufs=2) as rpool,
            tc.tile_pool(name="outp", bufs=1) as outpool,
            tc.tile_pool(name="ps", bufs=2, space="PSUM") as ps,
        ):
          def body():
            # ---- constant / input loads (gpsimd DMA casts f32 -> f32r) ----
            wq_sb = cpool.tile([128, 2, HID], F32R, tag="wq")
            nc.gpsimd.dma_start(wq_sb[:], wqT.rearrange("(kt p) m -> p kt m", p=128))
            wkv_sb = cpool.tile([128, 2, 2 * HID], F32R, tag="wkv")
            nc.gpsimd.dma_start(wkv_sb[:], wkvT.rearrange("(kt p) m -> p kt m", p=128))
            wout_sb = cpool.tile([128, 4, DIM], F32R, tag="wout")
            nc.gpsimd.dma_start(wout_sb[:], woutT.rearrange("(ct p) o -> p ct o", p=128))
            bout_sb = cpool.tile([128, 2], F32, tag="bout")
            nc.sync.dma_start(bout_sb[:], bout[:])

            # split x loads into chunks so the first projections unblock early
            xkv_sb = xpool.tile([128, 2, N], F32R, tag="xkv")
            xkv_r = x_kv.rearrange("(kt p) n -> p kt n", p=128)
            for c in range(4):
                nc.gpsimd.dma_start(xkv_sb[:, :, c * 512:(c + 1) * 512],
                                    xkv_r[:, :, c * 512:(c + 1) * 512])
            xq_sb = xpool.tile([128, 2, NH], F32R, tag="xq")
            xq_r = x_q.rearrange("(kt p) n -> p kt n", p=128)
            for c in range(2):
                nc.gpsimd.dma_start(xq_sb[:, :, c * 512:(c + 1) * 512],
                                    xq_r[:, :, c * 512:(c + 1) * 512])

            # ---- K projection: k_sb [d-major 512 rows, n 2048] ----
            # qkv/proj psum tiles share the "sim" tag slots (they are phase-
            # disjoint with attention) so the attnv accumulators can be
            # double-buffered within the 8 PSUM banks.
            k_sb = kqpool.tile([128, 4, N], F16, tag="k")
            for mt in range(4):
                for nt in range(4):
                    mm = ps.tile([128, 512], F32, tag="sim", name="mm")
                    for kt in range(2):
                        nc.tensor.matmul(
                            mm[:],
                            wkv_sb[:, kt, mt * 128:(mt + 1) * 128],
                            xkv_sb[:, kt, nt * 512:(nt + 1) * 512],
                            start=(kt == 0), stop=(kt == 1),
                        )
                    nc.vector.tensor_copy(
                        k_sb[:, mt, nt * 512:(nt + 1) * 512], mm[:])

            # ---- V^T projection (directly transposed) ----
            # vT[n, r] = sum_i x[i, n] * w_v[r, i]; lhsT = x n-tile, rhs = w_v^T
            vext = kqpool.tile([128, JT * H, DH + 1], F16, tag="vext")
            ones = cpool.tile([128, 1], F32, tag="ones")
            nc.gpsimd.memset(ones[:], 1.0)
            nc.vector.tensor_copy(
                vext[:, :, DH:DH + 1],
                ones[:, 0:1].to_broadcast([128, JT * H, 1]))
            for jt in range(JT):
                vt = ps.tile([128, 512], F32, tag="sim", name="vt")
                for kt in range(2):
                    nc.tensor.matmul(
                        vt[:],
                        xkv_sb[:, kt, jt * 128:(jt + 1) * 128],
                        wkv_sb[:, kt, HID:2 * HID],
                        start=(kt == 0), stop=(kt == 1),
                    )
                nc.vector.tensor_copy(
                    vext[:, jt * H:(jt + 1) * H, 0:DH],
                    vt[:].rearrange("p (h d) -> p h d", h=H))

            # ---- Q projection (query half only) ----
            q_sb = kqpool.tile([128, 4, NH], F16, tag="q")
            for mt in range(4):
                for nt in range(IC):
                    mm = ps.tile([128, 512], F32, tag="sim", name="mm")
                    for kt in range(2):
                        nc.tensor.matmul(
                            mm[:],
                            wq_sb[:, kt, mt * 128:(mt + 1) * 128],
                            xq_sb[:, kt, nt * 512:(nt + 1) * 512],
                            start=(kt == 0), stop=(kt == 1),
                        )
                    nc.vector.tensor_copy(q_sb[:, mt, nt * 512:(nt + 1) * 512], mm[:])

            # ---- attention ----
            outn = outpool.tile([128, 4, NH], F32R, tag="outn")
            ops = {}

            def norm(h):
                # outn = out * (1/denom), denom = row 64 of op
                hs = (h % 2) * DH
                op = ops.pop(h)
                for ic in range(IC):
                    rr = rpool.tile([1, 512], F32, tag="r")
                    nc.vector.reciprocal(rr[:], op[DH:DH + 1, ic * 512:(ic + 1) * 512])
                    rb = rpool.tile([DH, 512], F32, tag="rb")
                    nc.gpsimd.partition_broadcast(rb[:], rr[:])
                    nc.vector.tensor_mul(
                        outn[hs:hs + DH, h // 2, ic * 512:(ic + 1) * 512],
                        op[0:DH, ic * 512:(ic + 1) * 512],
                        rb[:],
                    )

            for h in range(H):
                hs = (h % 2) * DH
                for jt in range(JT):
                    sim = ps.tile([128, NH], F32, tag="sim")
                    for ic in range(IC):
                        nc.tensor.matmul(
                            sim[:, ic * 512:(ic + 1) * 512],
                            k_sb[hs:hs + DH, h // 2, jt * 128:(jt + 1) * 128],
                            q_sb[hs:hs + DH, h // 2, ic * 512:(ic + 1) * 512],
                            start=True, stop=True,
                        )
                    e = epool.tile([128, NH], F16, tag="E")
                    nc.scalar.activation(e[:], sim[:], AF.Exp, scale=SCALE)
                    if jt == 0:
                        ops[h] = ps.tile([DH + 1, NH], F32, tag="out", bufs=2,
                                         name=f"op{h}")
                    for ic in range(IC):
                        nc.tensor.matmul(
                            ops[h][:, ic * 512:(ic + 1) * 512],
                            vext[:, jt * H + h, :],
                            e[:, ic * 512:(ic + 1) * 512],
                            start=(jt == 0), stop=(jt == JT - 1),
                        )
                norm(h)

            # ---- output projection + bias ----
            y_sb = outpool.tile([128, 2, NH], F32, tag="y")
            for ot in range(2):
                for nt in range(IC):
                    yp = ps.tile([128, 512], F32, tag="sim", name="yp")
                    for ct in range(4):
                        nc.tensor.matmul(
                            yp[:],
                            wout_sb[:, ct, ot * 128:(ot + 1) * 128],
                            outn[:, ct, nt * 512:(nt + 1) * 512],
                            start=(ct == 0), stop=(ct == 3),
                        )
                    nc.vector.tensor_scalar_add(
                        y_sb[:, ot, nt * 512:(nt + 1) * 512], yp[:],
                        bout_sb[:, ot:ot + 1])
            nc.sync.dma_start(y.rearrange("(ot p) n -> p ot n", p=128), y_sb[:])

          if repeat == 1:
              body()
          else:
              with tc.For_i(0, repeat, 1):
                  body()

    nc.compile()
    return nc


def _make_in_maps(x, w_qkv, w_out, b_out):
    x = np.asarray(x, dtype=np.float32)
    w_qkv = np.asarray(w_qkv, dtype=np.float32)
    w_out = np.asarray(w_out, dtype=np.float32)
    b_out = np.asarray(b_out, dtype=np.float32)
    wqT = np.ascontiguousarray(w_qkv[0:HID].T)             # [256, 512]
    wkvT = np.ascontiguousarray(w_qkv[HID:3 * HID].T)      # [256, 1024]
    woutT = np.ascontiguousarray(w_out.T)                  # [512, 256]
    bout2 = np.ascontiguousarray(b_out.reshape(2, 128).T)  # [128, 2]
    maps = []
    for c in range(N_CORES):
        b, half = c // 2, c % 2
        maps.append({
            "x_kv": np.ascontiguousarray(x[b]),
            "x_q": np.ascontiguousarray(x[b][:, half * NH:(half + 1) * NH]),
            "wqT": wqT, "wkvT": wkvT, "woutT": woutT, "bout": bout2,
        })
    return maps


_NC_CACHE = None


def _get_nc():
    global _NC_CACHE
    if _NC_CACHE is None:
        _NC_CACHE = _build_nc(N_CORES)
    return _NC_CACHE


def kernel(x, w_qkv, w_out, b_out):
    in_maps = _make_in_maps(x, w_qkv, w_out, b_out)
    res = run_bass_kernel_spmd(_get_nc(), in_maps, list(range(N_CORES)))
    out = np.empty((B, DIM, N), dtype=np.float32)
    for c in range(N_CORES):
        b, half = c // 2, c % 2
        out[b][:, half * NH:(half + 1) * NH] = res.results[c]["y"]
    return out



# revision 2
# speedup vs baseline: 1.0579x; 1.0579x over previous
"""Trainium2 Bass kernel for nn_Attention_4329327034558.

Multi-head attention: x [4, 256, 2048], w_qkv [1536, 256], w_out [256, 512],
b_out [256] -> y [4, 256, 2048]  (8 heads, head dim 64).

Sharding over 8 NeuronCores: core c handles batch c//2 and query-half c%2
(all 8 heads). k/v are computed per core for the full sequence; q only for the
core's query half. Host side: transpose weights once, slice x per core, and
concatenate the two output halves per batch (no cross-core reduction needed).

Per-core device algorithm (attention matmuls in float16 — same accuracy as
float32r here but ~18% faster since fp16 weight loads use the fast path;
projections in float32r, with kt-outer loop order so each f32r stationary
tile is loaded once and reused across moving chunks — PE weight reloads
dominate the HW gap vs the cost model, and this reuse is worth ~10% wall
clock on hardware):
  k  = w_k @ x_b          [512, 2048]  (head-dim-major, heads stacked)
  vT = x_b^T @ w_v^T      [2048, 65*8] (produced directly transposed; a ones
                                        column is appended per head tile)
  q  = w_q @ x_b[:, half] [512, 1024]
  per head h, per key tile jt (128 keys):
    sim_T[j, i] = k_h^T q_h                   (PE, K=64 -> psum [128, 1024])
    E = exp(scale * sim_T)                    (ACT, psum -> sbuf f32r)
    [out_T | denom] += [v_h^T | 1]^T E        (PE, K=128, psum accum over jt;
                                               row 64 accumulates the softmax
                                               denominator for free)
  outn = out_T * (1/denom)   (DVE reciprocal + GPSIMD partition_broadcast +
                              DVE multiply; softmax max-subtraction is skipped:
                              logits are ~N(0,1) so exp() is safe in f32 and
                              mathematically identical to the reference)
  y_half = w_out @ concat_h(outn) + b_out     (PE + DVE bias-add)
"""

import numpy as np

import concourse.mybir as mybir
import concourse.tile as tile
from concourse import bacc
from concourse.bass_utils import run_bass_kernel_spmd

F32 = mybir.dt.float32
F32R = mybir.dt.float32r
F16 = mybir.dt.float16
AF = mybir.ActivationFunctionType

B = 4          # batch
DIM = 256      # channels
N = 2048       # sequence length
NH = 1024      # queries per core (n/2)
H = 8          # heads
DH = 64        # head dim
HID = 512      # h*dh
SCALE = DH ** -0.5
N_CORES = 8

JT = N // 128        # 16 key tiles
IC = NH // 512       # 2 query chunks


def _build_nc(num_devices=N_CORES, repeat=1):
    nc = bacc.Bacc("TRN2", target_bir_lowering=False, debug=False,
                   num_devices=num_devices)

    x_kv = nc.dram_tensor("x_kv", [DIM, N], F32, kind="ExternalInput")
    x_q = nc.dram_tensor("x_q", [DIM, NH], F32, kind="ExternalInput")
    wqT = nc.dram_tensor("wqT", [DIM, HID], F32, kind="ExternalInput")
    wkvT = nc.dram_tensor("wkvT", [DIM, 2 * HID], F32, kind="ExternalInput")
    woutT = nc.dram_tensor("woutT", [HID, DIM], F32, kind="ExternalInput")
    bout = nc.dram_tensor("bout", [128, 2], F32, kind="ExternalInput")
    y = nc.dram_tensor("y", [DIM, NH], F32, kind="ExternalOutput")

    with tile.TileContext(nc) as tc:
        with (
            tc.tile_pool(name="const", bufs=1) as cpool,
            tc.tile_pool(name="xin", bufs=1) as xpool,
            tc.tile_pool(name="kq", bufs=1) as kqpool,
            tc.tile_pool(name="epool", bufs=3) as epool,
            tc.tile_pool(name="rpool", bufs=2) as rpool,
            tc.tile_pool(name="outp", bufs=1) as outpool,
            tc.tile_pool(name="ps", bufs=2, space="PSUM") as ps,
        ):
          def body():
            # ---- constant / input loads (gpsimd DMA casts f32 -> f32r) ----
            wq_sb = cpool.tile([128, 2, HID], F32R, tag="wq")
            nc.gpsimd.dma_start(wq_sb[:], wqT.rearrange("(kt p) m -> p kt m", p=128))
            wkv_sb = cpool.tile([128, 2, 2 * HID], F32R, tag="wkv")
            nc.gpsimd.dma_start(wkv_sb[:], wkvT.rearrange("(kt p) m -> p kt m", p=128))
            wout_sb = cpool.tile([128, 4, DIM], F32R, tag="wout")
            nc.gpsimd.dma_start(wout_sb[:], woutT.rearrange("(ct p) o -> p ct o", p=128))
            bout_sb = cpool.tile([128, 2], F32, tag="bout")
            nc.sync.dma_start(bout_sb[:], bout[:])

            # split x loads into chunks so the first projections unblock early
            xkv_sb = xpool.tile([128, 2, N], F32R, tag="xkv")
            xkv_r = x_kv.rearrange("(kt p) n -> p kt n", p=128)
            for c in range(4):
                nc.gpsimd.dma_start(xkv_sb[:, :, c * 512:(c + 1) * 512],
                                    xkv_r[:, :, c * 512:(c + 1) * 512])
            xq_sb = xpool.tile([128, 2, NH], F32R, tag="xq")
            xq_r = x_q.rearrange("(kt p) n -> p kt n", p=128)
            for c in range(2):
                nc.gpsimd.dma_start(xq_sb[:, :, c * 512:(c + 1) * 512],
                                    xq_r[:, :, c * 512:(c + 1) * 512])

            # ---- K projection: k_sb [d-major 512 rows, n 2048] ----
            # qkv/proj psum tiles share the "sim" tag slots (they are phase-
            # disjoint with attention) so the attnv accumulators can be
            # double-buffered within the 8 PSUM banks.
            k_sb = kqpool.tile([128, 4, N], F16, tag="k")
            for mt in range(4):
                for ntp in range(2):
                    mms = [ps.tile([128, 512], F32, tag="sim", name=f"mm{s}")
                           for s in (0, 1)]
                    for kt in range(2):
                        for s in range(2):
                            nt = 2 * ntp + s
                            nc.tensor.matmul(
                                mms[s][:],
                                wkv_sb[:, kt, mt * 128:(mt + 1) * 128],
                                xkv_sb[:, kt, nt * 512:(nt + 1) * 512],
                                start=(kt == 0), stop=(kt == 1),
                            )
                    for s in range(2):
                        nt = 2 * ntp + s
                        nc.vector.tensor_copy(
                            k_sb[:, mt, nt * 512:(nt + 1) * 512], mms[s][:])

            # ---- V^T projection (directly transposed) ----
            # vT[n, r] = sum_i x[i, n] * w_v[r, i]; lhsT = x n-tile, rhs = w_v^T
            vext = kqpool.tile([128, JT * H, DH + 1], F16, tag="vext")
            ones = cpool.tile([128, 1], F32, tag="ones")
            nc.gpsimd.memset(ones[:], 1.0)
            nc.vector.tensor_copy(
                vext[:, :, DH:DH + 1],
                ones[:, 0:1].to_broadcast([128, JT * H, 1]))
            for jt in range(JT):
                vt = ps.tile([128, 512], F32, tag="sim", name="vt")
                for kt in range(2):
                    nc.tensor.matmul(
                        vt[:],
                        xkv_sb[:, kt, jt * 128:(jt + 1) * 128],
                        wkv_sb[:, kt, HID:2 * HID],
                        start=(kt == 0), stop=(kt == 1),
                    )
                nc.vector.tensor_copy(
                    vext[:, jt * H:(jt + 1) * H, 0:DH],
                    vt[:].rearrange("p (h d) -> p h d", h=H))

            # ---- Q projection (query half only) ----
            q_sb = kqpool.tile([128, 4, NH], F16, tag="q")
            for mt in range(4):
                mms = [ps.tile([128, 512], F32, tag="sim", name=f"qmm{s}")
                       for s in (0, 1)]
                for kt in range(2):
                    for nt in range(IC):
                        nc.tensor.matmul(
                            mms[nt][:],
                            wq_sb[:, kt, mt * 128:(mt + 1) * 128],
                            xq_sb[:, kt, nt * 512:(nt + 1) * 512],
                            start=(kt == 0), stop=(kt == 1),
                        )
                for nt in range(IC):
                    nc.vector.tensor_copy(q_sb[:, mt, nt * 512:(nt + 1) * 512],
                                          mms[nt][:])

            # ---- attention ----
            outn = outpool.tile([128, 4, NH], F32R, tag="outn")
            ops = {}

            def norm(h):
                # outn = out * (1/denom), denom = row 64 of op
                hs = (h % 2) * DH
                op = ops.pop(h)
                for ic in range(IC):
                    rr = rpool.tile([1, 512], F32, tag="r")
                    nc.vector.reciprocal(rr[:], op[DH:DH + 1, ic * 512:(ic + 1) * 512])
                    rb = rpool.tile([DH, 512], F32, tag="rb")
                    nc.gpsimd.partition_broadcast(rb[:], rr[:])
                    nc.vector.tensor_mul(
                        outn[hs:hs + DH, h // 2, ic * 512:(ic + 1) * 512],
                        op[0:DH, ic * 512:(ic + 1) * 512],
                        rb[:],
                    )

            for h in range(H):
                hs = (h % 2) * DH
                for jt in range(JT):
                    sim = ps.tile([128, NH], F32, tag="sim")
                    for ic in range(IC):
                        nc.tensor.matmul(
                            sim[:, ic * 512:(ic + 1) * 512],
                            k_sb[hs:hs + DH, h // 2, jt * 128:(jt + 1) * 128],
                            q_sb[hs:hs + DH, h // 2, ic * 512:(ic + 1) * 512],
                            start=True, stop=True,
                        )
                    e = epool.tile([128, NH], F16, tag="E")
                    nc.scalar.activation(e[:], sim[:], AF.Exp, scale=SCALE)
                    if jt == 0:
                        ops[h] = ps.tile([DH + 1, NH], F32, tag="out", bufs=2,
                                         name=f"op{h}")
                    for ic in range(IC):
                        nc.tensor.matmul(
                            ops[h][:, ic * 512:(ic + 1) * 512],
                            vext[:, jt * H + h, :],
                            e[:, ic * 512:(ic + 1) * 512],
                            start=(jt == 0), stop=(jt == JT - 1),
                        )
                norm(h)

            # ---- output projection + bias ----
            y_sb = outpool.tile([128, 2, NH], F32, tag="y")
            for ot in range(2):
                yps = [ps.tile([128, 512], F32, tag="sim", name=f"yp{s}")
                       for s in (0, 1)]
                for ct in range(4):
                    for nt in range(IC):
                        nc.tensor.matmul(
                            yps[nt][:],
                            wout_sb[:, ct, ot * 128:(ot + 1) * 128],
                            outn[:, ct, nt * 512:(nt + 1) * 512],
                            start=(ct == 0), stop=(ct == 3),
                        )
                for nt in range(IC):
                    nc.vector.tensor_scalar_add(
                        y_sb[:, ot, nt * 512:(nt + 1) * 512], yps[nt][:],
                        bout_sb[:, ot:ot + 1])
            nc.sync.dma_start(y.rearrange("(ot p) n -> p ot n", p=128), y_sb[:])

          if repeat == 1:
              body()
          else:
              with tc.For_i(0, repeat, 1):
                  body()

    nc.compile()
    return nc


def _make_in_maps(x, w_qkv, w_out, b_out):
    x = np.asarray(x, dtype=np.float32)
    w_qkv = np.asarray(w_qkv, dtype=np.float32)
    w_out = np.asarray(w_out, dtype=np.float32)
    b_out = np.asarray(b_out, dtype=np.float32)
    wqT = np.ascontiguousarray(w_qkv[0:HID].T)             # [256, 512]
    wkvT = np.ascontiguousarray(w_qkv[HID:3 * HID].T)      # [256, 1024]
    woutT = np.ascontiguousarray(w_out.T)                  # [512, 256]
    bout2 = np.ascontiguousarray(b_out.reshape(2, 128).T)  # [128, 2]
    maps = []
    for c in range(N_CORES):
        b, half = c // 2, c % 2
        maps.append({
            "x_kv": np.ascontiguousarray(x[b]),
            "x_q": np.ascontiguousarray(x[b][:, half * NH:(half + 1) * NH]),
            "wqT": wqT, "wkvT": wkvT, "woutT": woutT, "bout": bout2,
        })
    return maps


_NC_CACHE = None


def _get_nc():
    global _NC_CACHE
    if _NC_CACHE is None:
        _NC_CACHE = _build_nc(N_CORES)
    return _NC_CACHE


def kernel(x, w_qkv, w_out, b_out):
    in_maps = _make_in_maps(x, w_qkv, w_out, b_out)
    res = run_bass_kernel_spmd(_get_nc(), in_maps, list(range(N_CORES)))
    out = np.empty((B, DIM, N), dtype=np.float32)
    for c in range(N_CORES):
        b, half = c // 2, c % 2
        out[b][:, half * NH:(half + 1) * NH] = res.results[c]["y"]
    return out



# revision 3
# speedup vs baseline: 1.2393x; 1.1715x over previous
"""Trainium2 Bass kernel for nn_Attention_4329327034558.

Multi-head attention: x [4, 256, 2048], w_qkv [1536, 256], w_out [256, 512],
b_out [256] -> y [4, 256, 2048]  (8 heads, head dim 64).

Sharding over 8 NeuronCores: core c handles batch c//2 and query-half c%2
(all 8 heads). k/v are computed per core for the full sequence; q only for the
core's query half. Host side: transpose weights once, slice x per core, and
concatenate the two output halves per batch (no cross-core reduction needed).

Per-core device algorithm (attention matmuls in float16 — same accuracy as
float32r here but ~18% faster since fp16 weight loads use the fast path;
projections fed fp16 operands cast on the host — fp16 stationary loads are
much cheaper than f32r on the PE, and host casting avoids both on-chip
conversion and 2x the DMA bytes — with kt-outer loop order so each
stationary tile is loaded once and reused across moving chunks; the two
changes together are worth ~15% wall clock on hardware):
  k  = w_k @ x_b          [512, 2048]  (head-dim-major, heads stacked)
  vT = x_b^T @ w_v^T      [2048, 65*8] (produced directly transposed; a ones
                                        column is appended per head tile)
  q  = w_q @ x_b[:, half] [512, 1024]
  per head h, per key tile jt (128 keys):
    sim_T[j, i] = k_h^T q_h                   (PE, K=64 -> psum [128, 1024])
    E = exp(scale * sim_T)                    (ACT, psum -> sbuf f32r)
    [out_T | denom] += [v_h^T | 1]^T E        (PE, K=128, psum accum over jt;
                                               row 64 accumulates the softmax
                                               denominator for free)
  outn = out_T * (1/denom)   (DVE reciprocal + GPSIMD partition_broadcast +
                              DVE multiply; softmax max-subtraction is skipped:
                              logits are ~N(0,1) so exp() is safe in f32 and
                              mathematically identical to the reference)
  y_half = w_out @ concat_h(outn) + b_out     (PE + DVE bias-add)
"""

import numpy as np

import concourse.mybir as mybir
import concourse.tile as tile
from concourse import bacc
from concourse.bass_utils import run_bass_kernel_spmd

F32 = mybir.dt.float32
F32R = mybir.dt.float32r
F16 = mybir.dt.float16
AF = mybir.ActivationFunctionType

B = 4          # batch
DIM = 256      # channels
N = 2048       # sequence length
NH = 1024      # queries per core (n/2)
H = 8          # heads
DH = 64        # head dim
HID = 512      # h*dh
SCALE = DH ** -0.5
N_CORES = 8

JT = N // 128        # 16 key tiles
IC = NH // 512       # 2 query chunks


def _build_nc(num_devices=N_CORES, repeat=1):
    nc = bacc.Bacc("TRN2", target_bir_lowering=False, debug=False,
                   num_devices=num_devices)

    x_kv = nc.dram_tensor("x_kv", [DIM, N], F16, kind="ExternalInput")
    x_q = nc.dram_tensor("x_q", [DIM, NH], F16, kind="ExternalInput")
    wqT = nc.dram_tensor("wqT", [DIM, HID], F16, kind="ExternalInput")
    wkvT = nc.dram_tensor("wkvT", [DIM, 2 * HID], F16, kind="ExternalInput")
    woutT = nc.dram_tensor("woutT", [HID, DIM], F16, kind="ExternalInput")
    bout = nc.dram_tensor("bout", [128, 2], F32, kind="ExternalInput")
    y = nc.dram_tensor("y", [DIM, NH], F32, kind="ExternalOutput")

    with tile.TileContext(nc) as tc:
        with (
            tc.tile_pool(name="const", bufs=1) as cpool,
            tc.tile_pool(name="xin", bufs=1) as xpool,
            tc.tile_pool(name="kq", bufs=1) as kqpool,
            tc.tile_pool(name="epool", bufs=3) as epool,
            tc.tile_pool(name="rpool", bufs=2) as rpool,
            tc.tile_pool(name="outp", bufs=1) as outpool,
            tc.tile_pool(name="ps", bufs=2, space="PSUM") as ps,
        ):
          def body():
            # ---- constant / input loads (gpsimd DMA casts f32 -> f32r) ----
            wq_sb = cpool.tile([128, 2, HID], F16, tag="wq")
            nc.gpsimd.dma_start(wq_sb[:], wqT.rearrange("(kt p) m -> p kt m", p=128))
            wkv_sb = cpool.tile([128, 2, 2 * HID], F16, tag="wkv")
            nc.gpsimd.dma_start(wkv_sb[:], wkvT.rearrange("(kt p) m -> p kt m", p=128))
            wout_sb = cpool.tile([128, 4, DIM], F16, tag="wout")
            nc.gpsimd.dma_start(wout_sb[:], woutT.rearrange("(ct p) o -> p ct o", p=128))
            bout_sb = cpool.tile([128, 2], F32, tag="bout")
            nc.sync.dma_start(bout_sb[:], bout[:])

            # split x loads into chunks so the first projections unblock early
            xkv_sb = xpool.tile([128, 2, N], F16, tag="xkv")
            xkv_r = x_kv.rearrange("(kt p) n -> p kt n", p=128)
            for c in range(4):
                nc.gpsimd.dma_start(xkv_sb[:, :, c * 512:(c + 1) * 512],
                                    xkv_r[:, :, c * 512:(c + 1) * 512])
            xq_sb = xpool.tile([128, 2, NH], F16, tag="xq")
            xq_r = x_q.rearrange("(kt p) n -> p kt n", p=128)
            for c in range(2):
                nc.gpsimd.dma_start(xq_sb[:, :, c * 512:(c + 1) * 512],
                                    xq_r[:, :, c * 512:(c + 1) * 512])

            # ---- K projection: k_sb [d-major 512 rows, n 2048] ----
            # qkv/proj psum tiles share the "sim" tag slots (they are phase-
            # disjoint with attention) so the attnv accumulators can be
            # double-buffered within the 8 PSUM banks.
            k_sb = kqpool.tile([128, 4, N], F16, tag="k")
            for mt in range(4):
                for ntp in range(2):
                    mms = [ps.tile([128, 512], F32, tag="sim", name=f"mm{s}")
                           for s in (0, 1)]
                    for kt in range(2):
                        for s in range(2):
                            nt = 2 * ntp + s
                            nc.tensor.matmul(
                                mms[s][:],
                                wkv_sb[:, kt, mt * 128:(mt + 1) * 128],
                                xkv_sb[:, kt, nt * 512:(nt + 1) * 512],
                                start=(kt == 0), stop=(kt == 1),
                            )
                    for s in range(2):
                        nt = 2 * ntp + s
                        nc.vector.tensor_copy(
                            k_sb[:, mt, nt * 512:(nt + 1) * 512], mms[s][:])

            # ---- V^T projection (directly transposed) ----
            # vT[n, r] = sum_i x[i, n] * w_v[r, i]; lhsT = x n-tile, rhs = w_v^T
            vext = kqpool.tile([128, JT * H, DH + 1], F16, tag="vext")
            ones = cpool.tile([128, 1], F32, tag="ones")
            nc.gpsimd.memset(ones[:], 1.0)
            nc.vector.tensor_copy(
                vext[:, :, DH:DH + 1],
                ones[:, 0:1].to_broadcast([128, JT * H, 1]))
            for jt in range(JT):
                vt = ps.tile([128, 512], F32, tag="sim", name="vt")
                for kt in range(2):
                    nc.tensor.matmul(
                        vt[:],
                        xkv_sb[:, kt, jt * 128:(jt + 1) * 128],
                        wkv_sb[:, kt, HID:2 * HID],
                        start=(kt == 0), stop=(kt == 1),
                    )
                nc.vector.tensor_copy(
                    vext[:, jt * H:(jt + 1) * H, 0:DH],
                    vt[:].rearrange("p (h d) -> p h d", h=H))

            # ---- Q projection (query half only) ----
            q_sb = kqpool.tile([128, 4, NH], F16, tag="q")
            for mt in range(4):
                mms = [ps.tile([128, 512], F32, tag="sim", name=f"qmm{s}")
                       for s in (0, 1)]
                for kt in range(2):
                    for nt in range(IC):
                        nc.tensor.matmul(
                            mms[nt][:],
                            wq_sb[:, kt, mt * 128:(mt + 1) * 128],
                            xq_sb[:, kt, nt * 512:(nt + 1) * 512],
                            start=(kt == 0), stop=(kt == 1),
                        )
                for nt in range(IC):
                    nc.vector.tensor_copy(q_sb[:, mt, nt * 512:(nt + 1) * 512],
                                          mms[nt][:])

            # ---- attention ----
            outn = outpool.tile([128, 4, NH], F16, tag="outn")
            ops = {}

            def norm(h):
                # outn = out * (1/denom), denom = row 64 of op
                hs = (h % 2) * DH
                op = ops.pop(h)
                for ic in range(IC):
                    rr = rpool.tile([1, 512], F32, tag="r")
                    nc.vector.reciprocal(rr[:], op[DH:DH + 1, ic * 512:(ic + 1) * 512])
                    rb = rpool.tile([DH, 512], F32, tag="rb")
                    nc.gpsimd.partition_broadcast(rb[:], rr[:])
                    nc.vector.tensor_mul(
                        outn[hs:hs + DH, h // 2, ic * 512:(ic + 1) * 512],
                        op[0:DH, ic * 512:(ic + 1) * 512],
                        rb[:],
                    )

            for h in range(H):
                hs = (h % 2) * DH
                for jt in range(JT):
                    sim = ps.tile([128, NH], F32, tag="sim")
                    for ic in range(IC):
                        nc.tensor.matmul(
                            sim[:, ic * 512:(ic + 1) * 512],
                            k_sb[hs:hs + DH, h // 2, jt * 128:(jt + 1) * 128],
                            q_sb[hs:hs + DH, h // 2, ic * 512:(ic + 1) * 512],
                            start=True, stop=True,
                        )
                    e = epool.tile([128, NH], F16, tag="E")
                    nc.scalar.activation(e[:], sim[:], AF.Exp, scale=SCALE)
                    if jt == 0:
                        ops[h] = ps.tile([DH + 1, NH], F32, tag="out", bufs=2,
                                         name=f"op{h}")
                    for ic in range(IC):
                        nc.tensor.matmul(
                            ops[h][:, ic * 512:(ic + 1) * 512],
                            vext[:, jt * H + h, :],
                            e[:, ic * 512:(ic + 1) * 512],
                            start=(jt == 0), stop=(jt == JT - 1),
                        )
                norm(h)

            # ---- output projection + bias ----
            y_sb = outpool.tile([128, 2, NH], F32, tag="y")
            for ot in range(2):
                yps = [ps.tile([128, 512], F32, tag="sim", name=f"yp{s}")
                       for s in (0, 1)]
                for ct in range(4):
                    for nt in range(IC):
                        nc.tensor.matmul(
                            yps[nt][:],
                            wout_sb[:, ct, ot * 128:(ot + 1) * 128],
                            outn[:, ct, nt * 512:(nt + 1) * 512],
                            start=(ct == 0), stop=(ct == 3),
                        )
                for nt in range(IC):
                    nc.vector.tensor_scalar_add(
                        y_sb[:, ot, nt * 512:(nt + 1) * 512], yps[nt][:],
                        bout_sb[:, ot:ot + 1])
            nc.sync.dma_start(y.rearrange("(ot p) n -> p ot n", p=128), y_sb[:])

          if repeat == 1:
              body()
          else:
              with tc.For_i(0, repeat, 1):
                  body()

    nc.compile()
    return nc


def _make_in_maps(x, w_qkv, w_out, b_out):
    x = np.asarray(x, dtype=np.float32)
    w_qkv = np.asarray(w_qkv, dtype=np.float32)
    w_out = np.asarray(w_out, dtype=np.float32)
    b_out = np.asarray(b_out, dtype=np.float32)
    wqT = np.ascontiguousarray(w_qkv[0:HID].T.astype(np.float16))
    wkvT = np.ascontiguousarray(w_qkv[HID:3 * HID].T.astype(np.float16))
    woutT = np.ascontiguousarray(w_out.T.astype(np.float16))
    bout2 = np.ascontiguousarray(b_out.reshape(2, 128).T)  # [128, 2]
    maps = []
    for c in range(N_CORES):
        b, half = c // 2, c % 2
        maps.append({
            "x_kv": np.ascontiguousarray(x[b].astype(np.float16)),
            "x_q": np.ascontiguousarray(
                x[b][:, half * NH:(half + 1) * NH].astype(np.float16)),
            "wqT": wqT, "wkvT": wkvT, "woutT": woutT, "bout": bout2,
        })
    return maps


_NC_CACHE = None


def _get_nc():
    global _NC_CACHE
    if _NC_CACHE is None:
        _NC_CACHE = _build_nc(N_CORES)
    return _NC_CACHE


def kernel(x, w_qkv, w_out, b_out):
    in_maps = _make_in_maps(x, w_qkv, w_out, b_out)
    res = run_bass_kernel_spmd(_get_nc(), in_maps, list(range(N_CORES)))
    out = np.empty((B, DIM, N), dtype=np.float32)
    for c in range(N_CORES):
        b, half = c // 2, c % 2
        out[b][:, half * NH:(half + 1) * NH] = res.results[c]["y"]
    return out

